# revision 45
# baseline (speedup 1.0000x reference)
"""Trainium2 kernel for nn_AggrEncoder (segment-max + BN + 1x1 conv + fc).

Sharding: pure data-parallel over batch, 4 rows/core on 8 cores.

Host prep (sharding/layout only): per core, the 4 rows' 2048 (row, window)
pairs are sorted by valid-element count (descending); a pair's column is its
rank.  The payload ships as ONE fp16 region [128, T_tot] laid out in
"prefix slices": slice j (width N_j = #pairs with count > j) holds the
(j+1)-th element of each of the first N_j columns.  Total columns equal the
number of valid elements (masked elements are dropped; the reference's
zeros-init scatter-max makes zero padding semantically neutral), so the
device still streams every payload byte and performs the entire reduction +
matmul chain.  BN+conv+fc fold into one (128->8) affine W_eff/b_eff; the
bias is applied host-side during unshard (empty windows then fall out as
W_eff @ 0 + b_eff automatically).

Device per core, scheduled around the DMA stream (the memory roofline):
  1. Region DMA: the tiny slices plus a slice-0 prefix first (their small
     chained DVE ops and the 0-clamps run while the rest of slice 0
     streams), then each remaining slice solo, widest -> narrowest, so the
     DVE chain tracks deliveries with no end-of-stream backlog and only the
     smallest slice's op is exposed after the last byte lands.
  2. DVE in-place prefix max, each slice split into a hot piece [0, X1)
     and a rest piece: pieces in different column ranges are independent,
     so they pipeline back-to-back with no semaphore gaps and the hot
     chain that gates the exposed tail stays narrow.  tensor_tensor max
     runs in the 2x DVE perf mode on fp16.  The reference's zeros-init
     clamp commutes with max, so two early tensor_scalar_max ops (4x mode)
     in the pre-stream idle window cover all columns once.
  3. Matmul chunks: late chunk boundaries align to slice widths so each
     fp16 matmul fires immediately after its gating DVE piece; the 4
     earliest-ready chunks pack into PSUM bank A (partition offsets
     0/32/64/96) -> ACT evacuation -> ACT-queue store, hidden under the
     stream; the 3 latest pack into bank B -> DVE evacuation -> SP-queue
     store as the only exposed tail.
Host unshard: gather each (row, window) output column, add b_eff.
"""

import sys

import numpy as np

for _p in ("/opt/trn_rl_repo",):
    if _p not in sys.path:
        sys.path.insert(0, _p)

import concourse.bass as bass
import concourse.bacc as bacc
import concourse.mybir as mybir
from concourse import bass_utils
from concourse._compat import get_trn_type
from concourse.tile import TileContext

B, T, D, Tu, Dout, M = 32, 4096, 128, 512, 64, 8
NCORES = 8
RPC = B // NCORES  # rows per core
NP = RPC * Tu  # (row, window) pairs per core = 2048
BN_EPS = 1e-5
TINY = 150  # slices narrower than this process early, latency hidden

_CACHE = {}


def _plan(widths):
    """Shared layout plan.  Returns (tiny, rest, offsets, dmas, t_tot, P).
    Region layout: [tiny block | slice 0 | rest slices widest->narrowest].
    DMA ranges: tiny block + slice-0 prefix of P cols, remainder of slice 0,
    then one transfer per rest slice (final one padded to >= 256 cols)."""
    K = len(widths)
    tiny = [j for j in range(1, K) if widths[j] < TINY]
    rest = [j for j in range(1, K) if widths[j] >= TINY]
    order = tiny + [0] + rest
    offsets = np.empty(K, np.int64)
    pos = 0
    for j in order:
        offsets[j] = pos
        pos += widths[j]
    t_tot = pos

    head_end = int(offsets[0]) + widths[0]
    # split the head so the tiny ops and low-column clamp can run while the
    # rest of slice 0 streams; the first piece stays >= ~930 cols so the
    # second transfer is not HWDGE-cadence-gated
    tt = int(offsets[0])
    P = min(max(930 - tt, 256), widths[0])
    if rest and tt + P < head_end - 256:
        dmas = [(0, tt + P), (tt + P, head_end)]
    else:
        P = widths[0]
        dmas = [(0, head_end)]
    # every rest slice ships solo so the DVE chain tracks deliveries with
    # no end-of-stream backlog; slices < 256 cols merge into the previous
    # transfer (except the final one, which pads instead - see below)
    for i, j in enumerate(rest):
        lo, hi = int(offsets[j]), int(offsets[j] + widths[j])
        if widths[j] < 256 and dmas and i < len(rest) - 1 and dmas[-1][1] == lo:
            dmas[-1] = (dmas[-1][0], hi)
        else:
            dmas.append((lo, hi))
    # pad transfers under 256 cols (512 B) up to 256 to dodge the 2x DMA
    # small-element penalty; padding columns are zeros nothing reads
    pad_end = t_tot
    fixed = []
    for lo, hi in dmas:
        if hi - lo < 256 and hi == pad_end:
            hi = lo + 256
            pad_end = hi
        fixed.append((lo, hi))
    return tiny, rest, offsets, fixed, max(t_tot, pad_end), P


def _chunk_plan(widths, tiny, rest):
    """Chunk layout: list of (lo, hi, out, part) with out in {'a','b'}.
    Late chunk boundaries align to the narrowest slice widths so each chunk
    fires right after its gating DVE op; the early region splits into
    <=512-wide chunks.  The 4 earliest-ready chunks ship via outa (hidden
    under the stream), the 3 latest via outb (the exposed tail)."""
    m = len(rest)
    if m >= 5:
        lw = [widths[rest[m - 1]], widths[rest[m - 2]],
              widths[rest[m - 3]], widths[rest[m - 4]]]
        bounds = [0, lw[0], lw[1], lw[2], lw[3]]
        if all(b1 - b0 <= 512 for b0, b1 in zip(bounds[:-1], bounds[1:])):
            late = list(zip(bounds[:-1], bounds[1:]))
            early_lo = bounds[-1]
            n_early = -(-(NP - early_lo) // 512)
            step = -(-(NP - early_lo) // n_early)
            early = []
            lo = early_lo
            while lo < NP:
                early.append((lo, min(lo + step, NP)))
                lo += step
            chunks = []
            for lo, hi in early:
                chunks.append([lo, hi, "a", None])
            chunks.append([late[3][0], late[3][1], "a", None])
            for lo, hi in reversed(late[:3]):
                chunks.append([lo, hi, "b", None])
            a_parts = iter([0, 32, 64, 96])
            b_parts = iter([0, 32, 64, 96])
            for ch in chunks:
                ch[3] = next(a_parts) if ch[2] == "a" else next(b_parts)
            if len([c for c in chunks if c[2] == "a"]) <= 4 and \
               len([c for c in chunks if c[2] == "b"]) <= 4:
                return [tuple(c) for c in chunks]
    # fallback: 4 fixed chunks
    parts = [("a", 0), ("a", 32), ("b", 0), ("b", 32)]
    return [(q * 512, (q + 1) * 512, parts[q][0], parts[q][1])
            for q in range(4)]


def build_bass(profile=None):
    """Build the Bass module for a given slice-width profile (N_0=2048,
    N_1, ...).  With None, returns the most recently built module."""
    if profile is None:
        if "nc" in _CACHE:
            return _CACHE["nc"]
        raise ValueError("build_bass needs a profile before first kernel() call")

    widths = list(profile)
    tiny, rest, offsets, dmas, t_tot, P = _plan(widths)
    chain = tiny + rest  # DVE processing order
    chunks = _chunk_plan(widths, tiny, rest)
    acc0 = int(offsets[0])  # region column where the accumulator starts

    wa = max(hi - lo for lo, hi, o, p in chunks if o == "a")
    wb = max(hi - lo for lo, hi, o, p in chunks if o == "b")
    wa = max(wa, 256)  # >=512B innermost runs avoid the 2x DMA penalty
    wb = max(wb, 256)

    # Split each slice op into independent column-range pieces: the hot
    # range [0, X1) chains through every slice and gates the exposed tail,
    # so keeping its pieces narrow shortens the critical chain; mid/early
    # pieces fill DVE's data-wait gaps.  Ranges align with chunk bounds.
    m = len(rest)
    X1 = widths[rest[m - 1]] if m >= 2 else NP
    pieces = []  # (slice_id, a, b) in emission order
    for j in chain:
        w = widths[j]
        for a, b in ((0, min(w, X1)), (X1, w)):
            if b > a:
                pieces.append((j, a, b))

    # chunk is complete after the last piece that overlaps it
    need = [0] * len(chunks)
    for i, (j, a, b) in enumerate(pieces, 1):
        for q, ch in enumerate(chunks):
            if a < ch[1] and b > ch[0]:
                need[q] = i

    nc = bacc.Bacc(get_trn_type() or "TRN2", target_bir_lowering=False)

    region = nc.dram_tensor("region", [D, t_tot], mybir.dt.float16, kind="ExternalInput")
    wefft = nc.dram_tensor("wefft", [D, M], mybir.dt.float16, kind="ExternalInput")
    outa = nc.dram_tensor("outa", [128, wa], mybir.dt.bfloat16, kind="ExternalOutput")
    outb = nc.dram_tensor("outb", [96, wb], mybir.dt.bfloat16, kind="ExternalOutput")

    with TileContext(nc) as tc:
        with (
            tc.tile_pool(name="const", bufs=1) as cpool,
            tc.tile_pool(name="rpool", bufs=1) as rpool,
            tc.tile_pool(name="opool", bufs=1) as opool,
            tc.tile_pool(name="psum", bufs=1, space="PSUM") as ppool,
        ):
            weff_sb = cpool.tile([D, M], mybir.dt.float16, tag="weff")

            # weights issue after the first two region transfers so their
            # (tiny) transfer doesn't delay slice 1's delivery
            R = rpool.tile([D, t_tot], mybir.dt.float16, tag="R")
            wg = min(4, len(dmas) - 1)
            for g, (lo, hi) in enumerate(dmas):
                nc.sync.dma_start(R[:, lo:hi], region[:, lo:hi])
                if g == wg:
                    nc.sync.dma_start(weff_sb[:], wefft[:])

            pa = ppool.tile([D, 512], mybir.dt.float32, tag="pa")
            pb = ppool.tile([D, 512], mybir.dt.float32, tag="pb")
            psum = {"a": pa, "b": pb}
            nc.vector.memset(pa[:], 0.0)
            nc.vector.memset(pb[:], 0.0)

            # early ACT op pulls the activation-table load off the tail
            warm = cpool.tile([D, 1], mybir.dt.float32, tag="warm")
            nc.scalar.copy(warm[:], weff_sb[:, 0:1])

            outa_sb = opool.tile([128, wa], mybir.dt.bfloat16, tag="oa")
            outb_sb = opool.tile([96, wb], mybir.dt.bfloat16, tag="ob")

            done = set()

            def A(lo, hi):  # accumulator view in region coordinates
                return R[:, acc0 + lo:acc0 + hi]

            def finish_ready_chunks(i_done):
                for q, (lo, hi, out, part) in enumerate(chunks):
                    if q in done or need[q] > i_done:
                        continue
                    done.add(q)
                    w = hi - lo
                    nc.tensor.matmul(
                        psum[out][part:part + M, 0:w],
                        weff_sb[:],
                        A(lo, hi),
                        start=True,
                        stop=True,
                        tile_position=(0, part),
                    )
                a_set = {q for q, ch in enumerate(chunks) if ch[2] == "a"}
                if done >= a_set and "a" not in done:
                    done.add("a")
                    nc.scalar.copy(outa_sb[:], pa[:, 0:wa])
                    nc.scalar.dma_start(outa[:], outa_sb[:])

            n_tiny_pieces = sum(1 for j, a, b in pieces if j in set(tiny))
            clamp_i = max(n_tiny_pieces, 1 if pieces else 0)
            for q in range(len(need)):
                need[q] = max(need[q], clamp_i)  # no matmul before the clamp
            for i, (j, a, b) in enumerate(pieces, 1):
                lo = int(offsets[j])
                nc.vector.tensor_tensor(
                    A(a, b), A(a, b), R[:, lo + a:lo + b],
                    op=mybir.AluOpType.max,
                )
                if i == clamp_i:
                    # early 0-clamps (4x DVE mode) in the idle window; max
                    # keeps values >= 0 through every later op.  Split at P
                    # so the low part only needs the first head transfer.
                    nc.vector.tensor_scalar_max(A(0, min(P, NP)),
                                                A(0, min(P, NP)), 0.0)
                    if P < NP:
                        nc.vector.tensor_scalar_max(A(P, NP), A(P, NP), 0.0)
                finish_ready_chunks(i)
            if not pieces:
                nc.vector.tensor_scalar_max(A(0, min(P, NP)),
                                            A(0, min(P, NP)), 0.0)
                if P < NP:
                    nc.vector.tensor_scalar_max(A(P, NP), A(P, NP), 0.0)
            finish_ready_chunks(len(pieces))
            finish_ready_chunks(len(chain))

            nc.vector.tensor_scalar_add(outb_sb[:], pb[0:96, 0:wb], 0.0)
            nc.sync.dma_start(outb[:], outb_sb[:])

    if not nc.is_finalized():
        nc.finalize()
    _CACHE["nc"] = nc
    _CACHE["profile"] = tuple(widths)
    _CACHE["chunks"] = chunks
    return nc


def _host_prep(x, mask, tw_uniq, bn_gamma, bn_beta, bn_mean, bn_var,
               conv_w, conv_b, fc_w, fc_b):
    tw = x[:, :, 0]
    feats = x[:, :, 1:]
    u0 = tw_uniq[:, 0, 0]
    idx = np.clip((tw - u0[:, None]).astype(np.int32), 0, Tu - 1)  # (B, T)
    valid = mask[:, :, 0].astype(bool)

    # fold BN + conv + fc into one affine (done in f64, shipped as f16/f32)
    s = (bn_gamma.astype(np.float64)
         / np.sqrt(bn_var.astype(np.float64) + BN_EPS))
    t_aff = bn_beta.astype(np.float64) - bn_mean.astype(np.float64) * s
    wc = fc_w.astype(np.float64) @ conv_w.astype(np.float64)  # (8, 128)
    w_eff = wc * s[None, :]
    b_eff = (fc_w.astype(np.float64)
             @ (conv_w.astype(np.float64) @ t_aff + conv_b.astype(np.float64))
             + fc_b.astype(np.float64))
    wefft = np.ascontiguousarray(w_eff.T.astype(np.float16))  # (128, 8)
    beff = b_eff.astype(np.float32)  # (8,)

    counts = np.zeros((B, Tu), np.int64)
    occ = np.zeros((B, T), np.int64)  # occurrence index of element in its window
    for b in range(B):
        iv = idx[b][valid[b]]
        tv = np.nonzero(valid[b])[0]
        o = np.argsort(iv, kind="stable")
        si = iv[o]
        cnt = np.bincount(si, minlength=Tu)
        counts[b] = cnt
        starts = np.concatenate([[0], np.cumsum(cnt)[:-1]])
        occ[b, tv[o]] = np.arange(len(si)) - starts[si]

    core_counts = counts.reshape(NCORES, NP)  # pair = b_local * Tu + w
    ranks = np.empty((NCORES, NP), np.int64)
    for c in range(NCORES):
        ranks[c, np.argsort(-core_counts[c], kind="stable")] = np.arange(NP)

    kmax = int(counts.max())
    widths = [NP]
    for j in range(1, max(kmax, 1)):
        n = int((core_counts > j).sum(axis=1).max())
        if n <= 0:
            break
        widths.append(n)
    widths = tuple(widths)

    _, _, offsets, _, t_tot, _ = _plan(widths)

    regions = np.zeros((NCORES, D, t_tot), np.float16)
    for c in range(NCORES):
        rows = slice(c * RPC, (c + 1) * RPC)
        bl, tv = np.nonzero(valid[rows])
        w = idx[rows][bl, tv]
        j = occ[rows][bl, tv]
        pair = bl * Tu + w
        col = offsets[j] + ranks[c, pair]
        regions[c][:, col] = feats[rows][bl, tv].astype(np.float16).T

    return regions, widths, ranks, wefft, beff


def _unshard(res, ranks, beff, chunks):
    # per acc column: source (0=outb, 1=outa), partition base, column
    src = np.empty(NP, np.int64)
    pbase = np.empty(NP, np.int64)
    colof = np.empty(NP, np.int64)
    for lo, hi, out, part in chunks:
        src[lo:hi] = 1 if out == "a" else 0
        pbase[lo:hi] = part
        colof[lo:hi] = np.arange(hi - lo)

    final = np.empty((B, Tu, M), np.float32)
    for c in range(NCORES):
        EB = res.results[c]["outb"].astype(np.float32)
        EA = res.results[c]["outa"].astype(np.float32)
        r = ranks[c]
        s, pb_, co = src[r], pbase[r], colof[r]
        vals = np.where(
            (s == 0)[:, None],
            EB[np.minimum(pb_, EB.shape[0] - M)[:, None] + np.arange(M)[None, :],
               np.minimum(co, EB.shape[1] - 1)[:, None]],
            EA[np.minimum(pb_, EA.shape[0] - M)[:, None] + np.arange(M)[None, :],
               np.minimum(co, EA.shape[1] - 1)[:, None]],
        )
        final[c * RPC:(c + 1) * RPC] = (
            vals.reshape(RPC, Tu, M) + beff[None, None, :]
        )
    return final


def kernel(x, mask, tw_uniq, bn_gamma, bn_beta, bn_mean, bn_var,
           conv_w, conv_b, fc_w, fc_b):
    regions, profile, ranks, wefft, beff = _host_prep(
        x, mask, tw_uniq, bn_gamma, bn_beta, bn_mean, bn_var,
        conv_w, conv_b, fc_w, fc_b)

    if _CACHE.get("profile") != profile or "nc" not in _CACHE:
        _CACHE.pop("nc", None)
        build_bass(profile)
    nc = _CACHE["nc"]

    in_maps = [dict(region=regions[c], wefft=wefft) for c in range(NCORES)]
    res = bass_utils.run_bass_kernel_spmd(nc, in_maps, list(range(NCORES)))
    return _unshard(res, ranks, beff, _CACHE["chunks"])
